# revision 1
# baseline (speedup 1.0000x reference)
"""Trainium2 Bass kernel for nn_Block_29085518528833 (PVT-style pooling
attention block + IRB conv-MLP).

Sharding: 8 cores = 4 batches x 2 token-halves. Each core processes one
batch's full image for the (tiny, replicated) pooling/kv path, and a
4992-token range (4608 own + 384 halo) for the token-parallel paths.
The host permutes tokens so every core's own range is rows [0, 4992) --
a single uniform SPMD program, no cross-core communication.
"""

import sys

sys.path.insert(0, "/opt/trn_rl_repo")

from contextlib import ExitStack

import numpy as np
import ml_dtypes

import concourse.bass as bass
import concourse.bacc as bacc
import concourse.mybir as mybir
from concourse.tile import TileContext
from concourse.masks import make_identity

FP = mybir.dt.float32
FR = mybir.dt.float32r
BF = mybir.dt.bfloat16
AF = mybir.ActivationFunctionType
ALU = mybir.AluOpType

B = 4
C = 512
NH = 8
HD = 64
HID = 2048
HIMG = 96
NTOK = HIMG * HIMG
EPS = 1e-5
OHS = [8, 6, 5, 4]
LS = [o * o for o in OHS]
LOFF = [0, 64, 100, 125]
L = 141
AREAS = [144, 256, 400, 576]
L32 = [64, 64, 32, 32]  # LS padded to 32-multiples

HALF = NTOK // 2
HALO = 384
TRNG = HALF + HALO            # 4992
GRP = 384
NGRP = TRNG // GRP            # 13
NROWS = TRNG // HIMG          # 52
CCH = C // 128                # 4
MCH = HID // 128              # 16
NIMG_TILES = NTOK // 128      # 72
SCALE = HD ** (-0.5)

TAPS = [(di, dj) for di in (-1, 0, 1) for dj in (-1, 0, 1)]
CENTER = TAPS.index((0, 0))

_cache = {}


LOFF176 = [0, 64, 128, 160]


def _build_masks():
    M = np.zeros((NTOK, 176), np.float32)
    for s, oh in enumerate(OHS):
        sh = (np.arange(oh) * HIMG) // oh
        eh = -((-(np.arange(oh) + 1) * HIMG) // oh)
        for i in range(oh):
            for j in range(oh):
                hmask = np.zeros(HIMG, bool)
                hmask[sh[i]:eh[i]] = True
                wmask = np.zeros(HIMG, bool)
                wmask[sh[j]:eh[j]] = True
                tok = (hmask[:, None] & wmask[None, :]).reshape(-1)
                M[tok, LOFF176[s] + i * oh + j] = 1.0
    return M


def _conv_ranges(di, dj, nrows, ncols, lo_open, hi_open):
    oi0 = 0 if (di >= 0 or lo_open) else 1
    oi1 = nrows if (di <= 0 or hi_open) else nrows - 1
    oj0 = max(0, -dj)
    oj1 = ncols - max(0, dj)
    if oi1 <= oi0 or oj1 <= oj0:
        return None
    return oi0, oi1, oj0, oj1


def build_program():
    nc = bacc.Bacc("TRN2", target_bir_lowering=False, debug=False, num_devices=8)

    def din(name, shape, dtype=FP):
        return nc.dram_tensor(name, list(shape), dtype, kind="ExternalInput").ap()

    I = {}
    I["x"] = din("x", [NTOK, C])
    I["masks"] = din("masks", [NTOK, 176])
    I["qgwT"] = din("qgwT", [C, C])
    I["qb"] = din("qb", [128, CCH])
    I["kwT"] = din("kwT", [C, C])
    I["vwT"] = din("vwT", [C, C])
    I["projT"] = din("projT", [NH, HD, C])
    I["projb"] = din("projb", [1, C])
    I["fc1T6"] = din("fc1T6", [C, HID])
    I["fc1b6"] = din("fc1b6", [128, MCH])
    I["diag"] = din("diag", [MCH, 9, 128, 128], BF)
    I["convb6"] = din("convb6", [128, MCH])
    I["dvew"] = din("dvew", [128, MCH, 9])
    I["fc2T6"] = din("fc2T6", [HID, C], BF)
    I["fc2b"] = din("fc2b", [1, C], BF)
    I["g1rep"] = din("g1rep", [128, C])
    I["abrep"] = din("abrep", [128, 4, C])
    I["agrep"] = din("agrep", [128, C])
    I["btrep"] = din("btrep", [128, C])
    I["poolw"] = din("poolw", [128, CCH, 4, 9])
    I["poolabd"] = din("poolabd", [128, CCH, 4])
    I["v2init"] = din("v2init", [L, NH * 128])
    I["onescol"] = din("onescol", [L, NH * NH])
    I["repmask"] = din("repmask", [NH, C])
    I["ident"] = din("ident", [128, 128])
    I["onesr"] = din("onesr", [1, 128])

    out_dram = nc.dram_tensor("out", [TRNG, C], FP, kind="ExternalOutput").ap()
    x2_dram = nc.dram_tensor("x2_scratch", [TRNG, C], FP).ap()
    t_dram = nc.dram_tensor("t_scratch", [HID, TRNG], BF).ap()

    with TileContext(nc) as tc:
        _program(nc, tc, I, out_dram, x2_dram, t_dram)
    nc.compile()
    return nc


def _program(nc, tc, I, out_dram, x2_dram, t_dram):
    ctx = ExitStack()
    with ctx:
        consts = ctx.enter_context(tc.tile_pool(name="consts", bufs=1))
        persist = ctx.enter_context(tc.tile_pool(name="persist", bufs=1))
        small = ctx.enter_context(tc.tile_pool(name="small", bufs=4))

        ident = consts.tile([128, 128], FR)
        nc.sync.dma_start(out=ident[:], in_=I["ident"].bitcast(FR))
        eps_t = consts.tile([128, 1], FP)
        nc.vector.memset(eps_t, EPS)
        ones1r = consts.tile([1, 128], FR)
        nc.sync.dma_start(out=ones1r[:], in_=I["onesr"].bitcast(FR))
        ones1b = consts.tile([1, 128], BF)
        nc.vector.memset(ones1b, 1.0)

        qT_sb = [persist.tile([128, TRNG], BF, name=f"qT{m}") for m in range(CCH)]
        kT_sb = [persist.tile([128, L], BF, name=f"kT{m}") for m in range(CCH)]
        V_a = persist.tile([128, NH * 128], FR)
        V_b = persist.tile([13, NH * 128], FR)
        nc.sync.dma_start(out=V_a[:], in_=I["v2init"][0:128, :].bitcast(FR))
        nc.sync.dma_start(out=V_b[:], in_=I["v2init"][128:L, :].bitcast(FR))
        rhskv = [persist.tile([128, 144], FR, name=f"rhskv{m}") for m in range(CCH)]
        qb_sb = persist.tile([128, CCH], FP)
        nc.sync.dma_start(out=qb_sb[:], in_=I["qb"])
        projb_sb = persist.tile([1, C], FR)
        nc.sync.dma_start(out=projb_sb[:], in_=I["projb"].bitcast(FR))
        fc1b6 = persist.tile([128, MCH], FP)
        nc.sync.dma_start(out=fc1b6[:], in_=I["fc1b6"])
        convb6 = persist.tile([128, MCH], FP)
        nc.sync.dma_start(out=convb6[:], in_=I["convb6"])
        fc2b_sb = persist.tile([1, C], BF)
        nc.sync.dma_start(out=fc2b_sb[:], in_=I["fc2b"])
        dvew_sb = persist.tile([128, MCH, 9], FP)
        nc.sync.dma_start(out=dvew_sb[:], in_=I["dvew"])

        def ln_factors(xt, p):
            stats = small.tile([128, 6], FP, name="stats", tag="stats")
            nc.vector.bn_stats(out=stats[:p, :], in_=xt)
            mv = small.tile([128, 2], FP, name="mv", tag="mv")
            nc.vector.bn_aggr(out=mv[:p, :], in_=stats[:p, :])
            sd = small.tile([128, 1], FP, name="sd", tag="sd")
            nc.scalar.activation(out=sd[:p], in_=mv[:p, 1:2], func=AF.Sqrt,
                                 bias=eps_t[:p], scale=1.0)
            rs = small.tile([128, 1], FP, name="rs", tag="rs")
            nc.vector.reciprocal(out=rs[:p], in_=sd[:p])
            nmurs = small.tile([128, 1], FP, name="nmurs", tag="nmurs")
            nc.vector.tensor_mul(nmurs[:p], mv[:p, 0:1], rs[:p])
            nc.vector.tensor_scalar_mul(nmurs[:p], nmurs[:p], -1.0)
            return rs, nmurs

        spool = ctx.enter_context(tc.tile_pool(name="spool", bufs=1))

        # ============ PHASE A: LN1, pool sums, q^T ============
        s_sb = []
        with ExitStack() as phA:
            strA = phA.enter_context(tc.tile_pool(name="strA", bufs=3))
            wA = phA.enter_context(tc.tile_pool(name="wA", bufs=1))
            qgwT = [wA.tile([128, C], FR, name=f"qgwT{m}") for m in range(CCH)]
            for m in range(CCH):
                nc.sync.dma_start(out=qgwT[m][:],
                                  in_=I["qgwT"][m * 128:(m + 1) * 128, :].bitcast(FR))
            psA = phA.enter_context(tc.tile_pool(name="psA", bufs=1, space="PSUM"))
            spsA = psA.tile([128, C], FP, name="spsA")
            spsB = psA.tile([48, C], FP, name="spsB")
            # scale slices within the two banks (32-aligned bases)
            sps = [spsA[0:64], spsA[64:100], spsB[0:25], spsB[32:48]]
            psT = phA.enter_context(tc.tile_pool(name="psT", bufs=3, space="PSUM"))
            psQ = phA.enter_context(tc.tile_pool(name="psQ", bufs=3, space="PSUM"))
            xTg = phA.enter_context(tc.tile_pool(name="xTg", bufs=2))

            xT_cur = None
            for ti in range(NIMG_TILES):
                xt = strA.tile([128, C], FP, name="xt", tag="xt")
                nc.sync.dma_start(out=xt[:], in_=I["x"][ti * 128:(ti + 1) * 128, :])
                mt = strA.tile([128, 176], FR, name="mt", tag="mt")
                nc.sync.dma_start(out=mt[:],
                                  in_=I["masks"][ti * 128:(ti + 1) * 128, :].bitcast(FR))
                rs, nmurs = ln_factors(xt[:], 128)
                xh = strA.tile([128, C], FR, name="xh", tag="xh")
                nc.scalar.activation(out=xh[:], in_=xt[:], func=AF.Identity,
                                     bias=nmurs[:], scale=rs[:])
                nc.tensor.matmul(spsA[:], mt[:, 0:128], xh[:],
                                 start=(ti == 0), stop=(ti == NIMG_TILES - 1))
                nc.tensor.matmul(spsB[:], mt[:, 128:176], xh[:],
                                 start=(ti == 0), stop=(ti == NIMG_TILES - 1))
                if ti < TRNG // 128:
                    gi, sub = divmod(ti, 3)
                    if sub == 0:
                        xT_cur = [xTg.tile([128, GRP], FR, name=f"xT{cc}",
                                           tag=f"xT{cc}") for cc in range(CCH)]
                    for cc in range(CCH):
                        tp = psT.tile([128, 128], FR, name="tpA", tag="tpA")
                        nc.tensor.transpose(tp[:], xh[:, cc * 128:(cc + 1) * 128],
                                            ident[:])
                        nc.scalar.copy(xT_cur[cc][:, sub * 128:(sub + 1) * 128], tp[:])
                    if sub == 2:
                        for m in range(CCH):
                            qp = psQ.tile([128, GRP], FP, name="qp", tag="qp")
                            for cc in range(CCH):
                                nc.tensor.matmul(qp[:], qgwT[cc][:, m * 128:(m + 1) * 128],
                                                 xT_cur[cc][:], start=(cc == 0),
                                                 stop=(cc == CCH - 1))
                            nc.scalar.activation(
                                out=qT_sb[m][:, gi * GRP:(gi + 1) * GRP], in_=qp[:],
                                func=AF.Identity, bias=qb_sb[:, m:m + 1], scale=1.0)

            SBASE = [0, 64, 0, 32]
            for s in range(4):
                t = spool.tile([L32[s] + SBASE[s], C], FR, name=f"ssb{s}")
                nc.scalar.copy(t[SBASE[s]:SBASE[s] + LS[s], :], sps[s][:])
                s_sb.append(t)

        # ============ PHASE B: pool dwconv + attn LN + k/v ============
        with ExitStack() as phB:
            wB = phB.enter_context(tc.tile_pool(name="wB", bufs=1))
            kwT = [wB.tile([128, C], FR, name=f"kwT{m}") for m in range(CCH)]
            vwT = [wB.tile([128, C], FR, name=f"vwT{m}") for m in range(CCH)]
            for m in range(CCH):
                nc.sync.dma_start(out=kwT[m][:],
                                  in_=I["kwT"][m * 128:(m + 1) * 128, :].bitcast(FR))
                nc.sync.dma_start(out=vwT[m][:],
                                  in_=I["vwT"][m * 128:(m + 1) * 128, :].bitcast(FR))
            g1rep = wB.tile([128, C], FP, name="g1rep")
            nc.sync.dma_start(out=g1rep[:], in_=I["g1rep"])
            abrep = wB.tile([128, 4, C], FP, name="abrep")
            nc.sync.dma_start(out=abrep[:], in_=I["abrep"])
            agrep = wB.tile([128, C], FP, name="agrep")
            nc.sync.dma_start(out=agrep[:], in_=I["agrep"])
            btrep = wB.tile([128, C], FP, name="btrep")
            nc.sync.dma_start(out=btrep[:], in_=I["btrep"])
            poolw = wB.tile([128, CCH, 4, 9], FP, name="poolw")
            nc.sync.dma_start(out=poolw[:], in_=I["poolw"])
            poolabd = wB.tile([128, CCH, 4], FP, name="poolabd")
            nc.sync.dma_start(out=poolabd[:], in_=I["poolabd"])

            psB = phB.enter_context(tc.tile_pool(name="psB", bufs=2, space="PSUM"))
            sbB = phB.enter_context(tc.tile_pool(name="sbB", bufs=2))
            accP = phB.enter_context(tc.tile_pool(name="accP", bufs=1))

            SBASE = [0, 64, 0, 32]
            for s in range(4):
                b0 = SBASE[s]
                nc.vector.tensor_mul(s_sb[s][b0:b0 + LS[s], :],
                                     s_sb[s][b0:b0 + LS[s], :],
                                     g1rep[b0:b0 + LS[s], :])
                nc.vector.tensor_add(s_sb[s][b0:b0 + LS[s], :],
                                     s_sb[s][b0:b0 + LS[s], :],
                                     abrep[b0:b0 + LS[s], s, :])

            pn = []
            for s in range(4):
                oh = OHS[s]
                s1T = [sbB.tile([128, LS[s]], FR, name=f"s1T{s}_{cc}", tag=f"s1T{cc}")
                       for cc in range(CCH)]
                b0 = SBASE[s]
                for cc in range(CCH):
                    tp = psB.tile([128, 512], FR, name="tpB", tag="pb")
                    nc.tensor.transpose(tp[:, 0:L32[s]],
                                        s_sb[s][b0:b0 + L32[s],
                                                cc * 128:(cc + 1) * 128],
                                        ident[b0:b0 + L32[s], b0:b0 + L32[s]])
                    nc.scalar.copy(s1T[cc][:], tp[:, 0:LS[s]])
                acc = [accP.tile([128, L32[s]], FR, name=f"acc{s}_{cc}", tag=f"acc{cc}")
                       for cc in range(CCH)]
                for cc in range(CCH):
                    nc.vector.tensor_scalar_add(acc[cc][:, 0:LS[s]], s1T[cc][:],
                                                poolabd[:, cc, s:s + 1])
                    for tap, (di, dj) in enumerate(TAPS):
                        r = _conv_ranges(di, dj, oh, oh, False, False)
                        if r is None:
                            continue
                        oi0, oi1, oj0, oj1 = r
                        o_ap = acc[cc][:, 0:LS[s]].rearrange(
                            "p (i j) -> p i j", i=oh)[:, oi0:oi1, oj0:oj1]
                        i_ap = s1T[cc][:].rearrange("p (i j) -> p i j", i=oh)[
                            :, oi0 + di:oi1 + di, oj0 + dj:oj1 + dj]
                        nc.vector.scalar_tensor_tensor(
                            out=o_ap, in0=i_ap, scalar=poolw[:, cc, s, tap:tap + 1],
                            in1=o_ap, op0=ALU.mult, op1=ALU.add)
                q_s = sbB.tile([LS[s], C], FP, name=f"q_s{s}", tag="q_s")
                for cc in range(CCH):
                    tp = psB.tile([128, 512], FR, name="tpB2", tag="pb")
                    nc.tensor.transpose(tp[0:L32[s], 0:128], acc[cc][:], ident[:])
                    nc.scalar.copy(q_s[:, cc * 128:(cc + 1) * 128], tp[0:LS[s], 0:128])
                rs, nmurs = ln_factors(q_s[:], LS[s])
                pn_s = sbB.tile([L32[s], C], FR, name=f"pn{s}", tag="pn_s")
                nc.scalar.activation(out=pn_s[0:LS[s], :], in_=q_s[:],
                                     func=AF.Identity,
                                     bias=nmurs[0:LS[s]], scale=rs[0:LS[s]])
                nc.vector.tensor_mul(pn_s[0:LS[s], :], pn_s[0:LS[s], :],
                                     agrep[0:LS[s], :].bitcast(FR))
                nc.vector.tensor_add(pn_s[0:LS[s], :], pn_s[0:LS[s], :],
                                     btrep[0:LS[s], :].bitcast(FR))
                pn.append(pn_s)

            for s in range(4):
                for cc in range(CCH):
                    tp = psB.tile([128, 512], FR, name="tpB3", tag="pb")
                    nc.tensor.transpose(tp[:, 0:L32[s]], pn[s][:, cc * 128:(cc + 1) * 128],
                                        ident[0:L32[s], 0:L32[s]])
                    nc.scalar.copy(rhskv[cc][:, LOFF[s]:LOFF[s] + LS[s]], tp[:, 0:LS[s]])

            for m in range(CCH):
                kp = psB.tile([128, 144], FP, name="kp", tag="pb")
                for cc in range(CCH):
                    nc.tensor.matmul(kp[:], kwT[cc][:, m * 128:(m + 1) * 128],
                                     rhskv[cc][:], start=(cc == 0), stop=(cc == CCH - 1))
                nc.scalar.copy(kT_sb[m][:], kp[:, 0:L])
            vp = psB.tile([128, C], FP, name="vp", tag="pb")
            for cc in range(CCH):
                nc.tensor.matmul(vp[:], rhskv[cc][:, 0:128], vwT[cc][:],
                                 start=(cc == 0), stop=(cc == CCH - 1))
            for h in range(NH):
                nc.scalar.copy(V_a[:, h * 128:h * 128 + 64],
                               vp[:, h * 64:h * 64 + 64])
            vp2 = psB.tile([13, C], FP, name="vp2", tag="pb")
            for cc in range(CCH):
                nc.tensor.matmul(vp2[:], rhskv[cc][:, 128:L], vwT[cc][:],
                                 start=(cc == 0), stop=(cc == CCH - 1))
            for h in range(NH):
                nc.scalar.copy(V_b[:, h * 128:h * 128 + 64],
                               vp2[:, h * 64:h * 64 + 64])

        # ============ PHASE C: attention + proj + LN2 + fc1 + hswish ============
        with ExitStack() as phC:
            strC = phC.enter_context(tc.tile_pool(name="strC", bufs=4))
            wC = phC.enter_context(tc.tile_pool(name="wC", bufs=1))
            fc1T = [wC.tile([128, HID], FR, name=f"fc1T{cc}") for cc in range(CCH)]
            for cc in range(CCH):
                nc.sync.dma_start(out=fc1T[cc][:],
                                  in_=I["fc1T6"][cc * 128:(cc + 1) * 128, :].bitcast(FR))
            projT8 = []
            for h in range(NH):
                t = wC.tile([HD, C], FR, name=f"projT{h}")
                nc.sync.dma_start(out=t[:], in_=I["projT"][h].bitcast(FR))
                projT8.append(t)

            onescol_a = wC.tile([128, NH * NH], FR, name="onescol_a")
            nc.sync.dma_start(out=onescol_a[:], in_=I["onescol"][0:128, :].bitcast(FR))
            onescol_b = wC.tile([13, NH * NH], FR, name="onescol_b")
            nc.sync.dma_start(out=onescol_b[:], in_=I["onescol"][128:L, :].bitcast(FR))
            repmask = wC.tile([NH, C], FR, name="repmask")
            nc.sync.dma_start(out=repmask[:], in_=I["repmask"].bitcast(FR))
            psC = phC.enter_context(tc.tile_pool(name="psC", bufs=4, space="PSUM"))
            psDen = phC.enter_context(tc.tile_pool(name="psDen", bufs=2, space="PSUM"))
            psSb = phC.enter_context(tc.tile_pool(name="psSb", bufs=2, space="PSUM"))
            sbE = phC.enter_context(tc.tile_pool(name="sbE", bufs=4))
            sbA = phC.enter_context(tc.tile_pool(name="sbA", bufs=18))
            sbC = phC.enter_context(tc.tile_pool(name="sbC", bufs=2))
            sbT = phC.enter_context(tc.tile_pool(name="sbT", bufs=3))

            for g in range(NGRP):
                g0 = g * GRP
                den = psDen.tile([NH, GRP], FP, name="den", tag="den")
                A_h = []
                for h in range(NH):
                    m, hh = h // 2, (h % 2) * 64
                    Sa = psC.tile([128, C], FP, name="Sa", tag="pc")
                    nc.tensor.matmul(Sa[:, 0:GRP], kT_sb[m][hh:hh + 64, 0:128],
                                     qT_sb[m][hh:hh + 64, g0:g0 + GRP],
                                     start=True, stop=True)
                    Sb = psSb.tile([13, GRP], FP, name="Sb", tag="Sb")
                    nc.tensor.matmul(Sb[:], kT_sb[m][hh:hh + 64, 128:L],
                                     qT_sb[m][hh:hh + 64, g0:g0 + GRP],
                                     start=True, stop=True)
                    Ea = sbE.tile([128, GRP], FR, name="Ea", tag="Ea")
                    nc.scalar.activation(out=Ea[:], in_=Sa[:, 0:GRP], func=AF.Exp)
                    Eb = sbE.tile([13, GRP], FR, name="Eb", tag="Eb")
                    nc.scalar.activation(out=Eb[:], in_=Sb[:], func=AF.Exp)
                    nc.tensor.matmul(den[:], onescol_a[:, h * 8:h * 8 + 8], Ea[:],
                                     start=(h == 0), stop=False)
                    nc.tensor.matmul(den[:], onescol_b[:, h * 8:h * 8 + 8], Eb[:],
                                     start=False, stop=(h == NH - 1))
                    Uh = psC.tile([128, C], FP, name="Uh", tag="pc")
                    nc.tensor.matmul(Uh[0:64, 0:GRP], V_a[:, h * 128:h * 128 + 64],
                                     Ea[:], start=True, stop=False)
                    nc.tensor.matmul(Uh[0:64, 0:GRP], V_b[:, h * 128:h * 128 + 64],
                                     Eb[:], start=False, stop=True)
                    Ah = sbA.tile([64, GRP], FR, name="Ah", tag="Ah")
                    nc.scalar.copy(Ah[:], Uh[0:64, 0:GRP])
                    A_h.append(Ah)
                recip = sbC.tile([NH, GRP], FR, name="recip", tag="recip")
                with nc.allow_low_precision("f32r reciprocal feeds matmul"):
                    nc.vector.reciprocal(recip[:], den[:])
                for h in range(NH):
                    rr = psC.tile([64, C], FP, name="rr", tag="pc")
                    nc.tensor.matmul(rr[:, 0:GRP], repmask[:, h * 64:h * 64 + 64],
                                     recip[:], start=True, stop=True)
                    nc.vector.tensor_mul(A_h[h][:], A_h[h][:], rr[:, 0:GRP])
                xh2T = [sbT.tile([128, GRP], FR, name=f"xh2T{cc}", tag=f"xh2T{cc}")
                        for cc in range(CCH)]
                for sub in range(3):
                    r0 = g0 + sub * 128
                    xp = psC.tile([128, C], FP, name="xp", tag="pc")
                    for h in range(NH):
                        nc.tensor.matmul(xp[:], A_h[h][:, sub * 128:(sub + 1) * 128],
                                         projT8[h][:], start=(h == 0), stop=False)
                    nc.tensor.matmul(xp[:], ones1r[:], projb_sb[:],
                                     start=False, stop=True)
                    xt2 = strC.tile([128, C], FP, name="xt2", tag="xt2")
                    nc.sync.dma_start(out=xt2[:], in_=I["x"][r0:r0 + 128, :])
                    x2s = strC.tile([128, C], FP, name="x2s", tag="x2s")
                    nc.vector.tensor_add(x2s[:], xt2[:], xp[:])
                    nc.sync.dma_start(out=x2_dram[r0:r0 + 128, :], in_=x2s[:])
                    rs, nmurs = ln_factors(x2s[:], 128)
                    xh2 = strC.tile([128, C], FR, name="xh2", tag="xh2")
                    nc.scalar.activation(out=xh2[:], in_=x2s[:], func=AF.Identity,
                                         bias=nmurs[:], scale=rs[:])
                    for cc in range(CCH):
                        tp = psC.tile([128, C], FR, name="tpC", tag="pc")
                        nc.tensor.transpose(tp[:, 0:128],
                                            xh2[:, cc * 128:(cc + 1) * 128], ident[:])
                        nc.scalar.copy(xh2T[cc][:, sub * 128:(sub + 1) * 128],
                                       tp[:, 0:128])
                for m in range(MCH):
                    fp = psC.tile([128, C], FP, name="fp", tag="pc")
                    for cc in range(CCH):
                        nc.tensor.matmul(fp[:, 0:GRP], fc1T[cc][:, m * 128:(m + 1) * 128],
                                         xh2T[cc][:], start=(cc == 0),
                                         stop=(cc == CCH - 1))
                    u = strC.tile([128, GRP], BF, name="u", tag="u")
                    nc.scalar.activation(out=u[:], in_=fp[:, 0:GRP], func=AF.Identity,
                                         bias=fc1b6[:, m:m + 1], scale=1.0)
                    c1 = strC.tile([128, GRP], BF, name="c1", tag="c1")
                    nc.vector.tensor_scalar(out=c1[:], in0=u[:], scalar1=0.5,
                                            scalar2=0.0, op0=ALU.add, op1=ALU.max)
                    tt = strC.tile([128, GRP], BF, name="tt", tag="tt")
                    nc.vector.scalar_tensor_tensor(out=tt[:], in0=c1[:], scalar=1.0,
                                                   in1=u[:], op0=ALU.min, op1=ALU.mult)
                    nc.sync.dma_start(
                        out=t_dram[m * 128:(m + 1) * 128, g0:g0 + GRP], in_=tt[:])

        # ============ PHASE E: dwconv + hswish + fc2 + residual ============
        with ExitStack() as phE:
            strE = phE.enter_context(tc.tile_pool(name="strE", bufs=4))
            wE = phE.enter_context(tc.tile_pool(name="wE", bufs=1))
            diag_sb = [wE.tile([128, 9, 128], BF, name=f"diag{m}") for m in range(MCH)]
            for m in range(MCH):
                src = bass.AP(tensor=I["diag"].tensor,
                              offset=I["diag"].offset + m * 9 * 128 * 128,
                              ap=[[128, 128], [128 * 128, 9], [1, 128]])
                nc.sync.dma_start(out=diag_sb[m][:], in_=src)
            fc2T = [wE.tile([128, C], BF, name=f"fc2T{m}") for m in range(MCH)]
            for m in range(MCH):
                nc.sync.dma_start(out=fc2T[m][:],
                                  in_=I["fc2T6"][m * 128:(m + 1) * 128, :])
            psD = phE.enter_context(tc.tile_pool(name="psD", bufs=3, space="PSUM"))
            psO = phE.enter_context(tc.tile_pool(name="psO", bufs=3, space="PSUM"))
            tch_p = phE.enter_context(tc.tile_pool(name="tch", bufs=34))
            t2_p = phE.enter_context(tc.tile_pool(name="t2", bufs=34))

            for g in range(NGRP):
                g0 = g * GRP
                grow = g * 4
                row0 = max(0, grow - 1)
                row1 = min(NROWS, grow + 5)
                ncr = row1 - row0
                tch = []
                for m in range(MCH):
                    t = tch_p.tile([128, 6 * HIMG], BF, name=f"tch{m}", tag="tch")
                    nc.sync.dma_start(
                        out=t[:, 0:ncr * HIMG],
                        in_=t_dram[m * 128:(m + 1) * 128, row0 * HIMG:row1 * HIMG])
                    tch.append(t)
                DVET = (TAPS.index((-1, 0)), TAPS.index((1, 0)))
                t2 = []
                for m in range(MCH):
                    dw = psD.tile([128, GRP], FP, name="dw", tag="dw")
                    taps = []
                    for tap, (di, dj) in enumerate(TAPS):
                        r = _conv_ranges(di, dj, 4, HIMG, grow > 0, grow + 4 < NROWS)
                        if r is not None:
                            taps.append((tap, di, dj, r))
                    pe_taps = [t_ for t_ in taps if t_[0] not in DVET]
                    dve_taps = [t_ for t_ in taps if t_[0] in DVET]
                    pe_taps.sort(key=lambda t_: t_[0] != CENTER)
                    for idx, (tap, di, dj, (oi0, oi1, oj0, oj1)) in enumerate(pe_taps):
                        o_ap = dw[:].rearrange("p (i j) -> p i j", i=4)[
                            :, oi0:oi1, oj0:oj1]
                        i_ap = tch[m][:].rearrange("p (i j) -> p i j", i=6)[
                            :, oi0 + di + (grow - row0):oi1 + di + (grow - row0),
                            oj0 + dj:oj1 + dj]
                        nc.tensor.matmul(o_ap, diag_sb[m][:, tap, :], i_ap,
                                         start=(idx == 0),
                                         stop=(idx == len(pe_taps) - 1))
                    # row-shift taps on DVE, contiguous full-width slices
                    acc = strE.tile([128, GRP], BF, name="acc", tag="acc")
                    first = True
                    for tap, di, dj, (oi0, oi1, oj0, oj1) in dve_taps:
                        o0 = oi0 * HIMG
                        o1 = oi1 * HIMG
                        i0 = (oi0 + di + (grow - row0)) * HIMG
                        if first and (oi0 != 0 or oi1 != 4):
                            nc.vector.memset(acc[:], 0.0)
                            first = False
                        if first:
                            nc.vector.tensor_scalar_mul(
                                acc[:, o0:o1], tch[m][:, i0:i0 + (o1 - o0)],
                                dvew_sb[:, m, tap:tap + 1])
                            first = False
                        else:
                            nc.vector.scalar_tensor_tensor(
                                out=acc[:, o0:o1], in0=tch[m][:, i0:i0 + (o1 - o0)],
                                scalar=dvew_sb[:, m, tap:tap + 1],
                                in1=acc[:, o0:o1], op0=ALU.mult, op1=ALU.add)
                    # p = (psum + conv_b/6) + acc
                    p = strE.tile([128, GRP], BF, name="p", tag="p")
                    nc.vector.scalar_tensor_tensor(out=p[:], in0=dw[:],
                                                   scalar=convb6[:, m:m + 1],
                                                   in1=acc[:], op0=ALU.add,
                                                   op1=ALU.add)
                    c2 = strE.tile([128, GRP], BF, name="c2", tag="c2")
                    nc.vector.tensor_scalar(out=c2[:], in0=p[:], scalar1=0.5,
                                            scalar2=0.0, op0=ALU.add, op1=ALU.max)
                    t2m = t2_p.tile([128, GRP], BF, name=f"t2_{m}", tag="t2")
                    nc.vector.scalar_tensor_tensor(out=t2m[:], in0=c2[:], scalar=1.0,
                                                   in1=p[:], op0=ALU.min, op1=ALU.mult)
                    t2.append(t2m)
                for sub in range(3):
                    r0 = g0 + sub * 128
                    op = psO.tile([128, C], FP, name="op", tag="op")
                    for m in range(MCH):
                        nc.tensor.matmul(op[:], t2[m][:, sub * 128:(sub + 1) * 128],
                                         fc2T[m][:], start=(m == 0), stop=False)
                    nc.tensor.matmul(op[:], ones1b[:], fc2b_sb[:],
                                     start=False, stop=True)
                    x2t = strE.tile([128, C], FP, name="x2t", tag="x2t")
                    nc.sync.dma_start(out=x2t[:], in_=x2_dram[r0:r0 + 128, :])
                    osb = strE.tile([128, C], FP, name="osb", tag="osb")
                    nc.vector.tensor_add(osb[:], x2t[:], op[:])
                    nc.sync.dma_start(out=out_dram[r0:r0 + 128, :], in_=osb[:])


def _host_prep(inputs):
    x = np.asarray(inputs["x"], np.float32)
    g1 = np.asarray(inputs["norm1_g"], np.float32)
    b1 = np.asarray(inputs["norm1_b"], np.float32)
    q_w = np.asarray(inputs["q_w"], np.float32)
    kv_w = np.asarray(inputs["kv_w"], np.float32)
    ag = np.asarray(inputs["attn_norm_g"], np.float32)
    ab = np.asarray(inputs["attn_norm_b"], np.float32)
    proj_w = np.asarray(inputs["proj_w"], np.float32)
    proj_b = np.asarray(inputs["proj_b"], np.float32)
    dconv_w = np.asarray(inputs["dconv_w"], np.float32)
    dconv_b = np.asarray(inputs["dconv_b"], np.float32)
    g2 = np.asarray(inputs["norm2_g"], np.float32)
    b2 = np.asarray(inputs["norm2_b"], np.float32)
    fc1_w = np.asarray(inputs["fc1_w"], np.float32)
    fc1_b = np.asarray(inputs["fc1_b"], np.float32)
    conv_w = np.asarray(inputs["conv_w"], np.float32)
    conv_b = np.asarray(inputs["conv_b"], np.float32)
    fc2_w = np.asarray(inputs["fc2_w"], np.float32)
    fc2_b = np.asarray(inputs["fc2_b"], np.float32)

    M = _build_masks()

    qgw = (q_w * g1[None, :]) * SCALE
    qgwT = np.ascontiguousarray(qgw.T)
    qb = np.ascontiguousarray(((q_w @ b1) * SCALE).reshape(CCH, 128).T)
    kwT = np.ascontiguousarray(kv_w[0:C].T)
    vwT = np.ascontiguousarray(kv_w[C:2 * C].T)
    projT = np.ascontiguousarray(proj_w.T.reshape(NH, HD, C))
    projb = proj_b.reshape(1, C)
    fc1T6 = np.ascontiguousarray(((fc1_w * g2[None, :]) / 6.0).T)
    fc1b6 = np.ascontiguousarray(((fc1_b + fc1_w @ b2) / 6.0).reshape(MCH, 128).T)
    diag = np.zeros((MCH, 9, 128, 128), np.float32)
    for m in range(MCH):
        for tap in range(9):
            di, dj = TAPS[tap]
            np.fill_diagonal(diag[m, tap],
                             conv_w[m * 128:(m + 1) * 128, 0, di + 1, dj + 1])
    diag = diag.astype(ml_dtypes.bfloat16)
    convb6 = np.ascontiguousarray((conv_b / 6.0).reshape(MCH, 128).T)
    dvew = np.zeros((128, MCH, 9), np.float32)
    for m in range(MCH):
        for tap in range(9):
            di, dj = TAPS[tap]
            dvew[:, m, tap] = conv_w[m * 128:(m + 1) * 128, 0, di + 1, dj + 1]
    fc2T6 = np.ascontiguousarray((fc2_w * 6.0).T).astype(ml_dtypes.bfloat16)
    fc2b = fc2_b.reshape(1, C).astype(ml_dtypes.bfloat16)
    g1rep = np.broadcast_to(g1, (128, C)).copy()
    abrep = np.stack([np.broadcast_to(AREAS[s] * b1, (128, C))
                      for s in range(4)], axis=1).copy()
    agrep = np.broadcast_to(ag, (128, C)).copy()
    btrep = np.broadcast_to(ab, (128, C)).copy()
    poolw = np.zeros((128, CCH, 4, 9), np.float32)
    poolabd = np.zeros((128, CCH, 4), np.float32)
    for cc in range(CCH):
        for s in range(4):
            for tap in range(9):
                di, dj = TAPS[tap]
                poolw[:, cc, s, tap] = dconv_w[s, cc * 128:(cc + 1) * 128, 0,
                                               di + 1, dj + 1]
            poolabd[:, cc, s] = AREAS[s] * dconv_b[s, cc * 128:(cc + 1) * 128]
    v2init = np.zeros((L, NH * 128), np.float32)
    for h in range(NH):
        v2init[:, h * 128 + 64:h * 128 + 128] = 1.0
    onescol = np.zeros((L, NH * NH), np.float32)
    for h in range(NH):
        onescol[:, h * 8 + h] = 1.0
    repmask = np.zeros((NH, C), np.float32)
    for h in range(NH):
        repmask[h, h * 64:(h + 1) * 64] = 1.0

    shared = dict(qgwT=qgwT, qb=qb, kwT=kwT, vwT=vwT, projT=projT, projb=projb,
                  fc1T6=fc1T6, fc1b6=fc1b6, diag=diag, convb6=convb6,
                  fc2T6=fc2T6, fc2b=fc2b, g1rep=g1rep, abrep=abrep, dvew=dvew,
                  agrep=agrep, btrep=btrep, poolw=poolw, poolabd=poolabd,
                  v2init=v2init, onescol=onescol, repmask=repmask,
                  ident=np.eye(128, dtype=np.float32),
                  onesr=np.ones((1, 128), np.float32))

    perms = []
    for half in range(2):
        f0 = 0 if half == 0 else NTOK - TRNG
        perms.append(np.concatenate([np.arange(f0, f0 + TRNG),
                                     np.arange(0, f0),
                                     np.arange(f0 + TRNG, NTOK)]))
    masks_p = [np.ascontiguousarray(M[p]) for p in perms]

    in_maps = []
    for b in range(B):
        for half in range(2):
            m = dict(shared)
            m["x"] = np.ascontiguousarray(x[b][perms[half]])
            m["masks"] = masks_p[half]
            in_maps.append(m)
    return in_maps


def kernel(**inputs):
    if "nc" not in _cache:
        _cache["nc"] = build_program()
    nc = _cache["nc"]

    from concourse.bass_utils import run_bass_kernel_spmd

    in_maps = _host_prep(inputs)
    core_ids = list(range(8))
    res = run_bass_kernel_spmd(nc, in_maps, core_ids)

    x = np.asarray(inputs["x"], np.float32)
    out = np.empty_like(x)
    for b in range(B):
        o0 = res.results[2 * b]["out"]
        o1 = res.results[2 * b + 1]["out"]
        out[b, 0:HALF] = o0[0:HALF]
        out[b, HALF:] = o1[HALO:]
    return out

